# revision 35
# baseline (speedup 1.0000x reference)
"""Trainium2 Bass kernel: 2-layer LSTM (Keras gate order i,f,g,o) + linear head.

Model: B=256, T=256, F=32, H=512, OUT=1.
Sharding: data-parallel over batch across 8 NeuronCores (32 rows/core);
LSTM weights replicated; hidden state device-local across the scan.

v2 layout (gate-permuted fold, bf16 matmuls, 4-way col-tiling):
  PSUM z tile [128, 512] fp32: partition p = 32*j + b  (j = H-slice 0..3,
  b = batch 0..31), column c = g*128 + h'  (g = gate i/f/g/o, h' = hidden
  position within slice j). Produced by 4 concurrent col-tiled matmuls
  (tile_position=(0,32j)), stationary = bf16 h^T chunks [128,32], moving =
  bf16 permuted weights [128,512].
  Gates then live on full 128 partitions: sigmoid/tanh on column ranges.
  State c/h folded [128,128] (p=32j+b, col h').  h^T for the next step's
  stationary comes from 4 PE-mode transposes of the folded h
  ([32,128] -> [128,32]) with an fp32->bf16 cast on the PSUM->SBUF copy.
"""

import numpy as np
import ml_dtypes

import concourse.bacc as bacc
import concourse.mybir as mybir
import concourse.tile as tile
from concourse.bass_utils import run_bass_kernel_spmd

B, T, F = 256, 256, 32
H = 512
OUT = 1
NCORES = 8
BL = B // NCORES          # 32 batch rows per core
KC = H // 128             # 4 K-chunks of 128 for H-contractions
NJ = 4                    # 4 col-tile groups (H-slices)

FP32 = mybir.dt.float32
BF16 = mybir.dt.bfloat16
AFT = mybir.ActivationFunctionType


def _build(Tsteps: int, has_b0: bool, has_b1: bool,
           reps: int = 1, ablate: str = "", tmode: str = "t4"):
    """Build the per-core SPMD program. Returns the Bacc object.

    reps>1 repeats the whole scan (state carried over) for timing contrast.
    ablate: "" | "notrans" (constant hT, skip transposes) |
            "gemm" (also skip gate math; sink-read PSUM).
    """
    nc = bacc.Bacc("TRN2", target_bir_lowering=False, debug=False)

    # ---- DRAM I/O (host pre-permuted, bf16 where matmul operands) ----
    xT_d = nc.dram_tensor("xT", [F, Tsteps * BL], BF16, kind="ExternalInput")
    U0_d = nc.dram_tensor("U0p", [128, KC * 2048], BF16, kind="ExternalInput")
    W1_d = nc.dram_tensor("W1p", [128, KC * 2048], BF16, kind="ExternalInput")
    U1_d = nc.dram_tensor("U1p", [128, KC * 2048], BF16, kind="ExternalInput")
    W0_d = nc.dram_tensor("W0p", [F, 2048], BF16, kind="ExternalInput")
    Wd_d = nc.dram_tensor("Wdr", [128, KC], BF16, kind="ExternalInput")
    id_d = nc.dram_tensor("ident128", [128, 128], FP32, kind="ExternalInput")
    b0_d = nc.dram_tensor("b0p", [1, 2048], BF16, kind="ExternalInput") if has_b0 else None
    b1_d = nc.dram_tensor("b1p", [1, 2048], BF16, kind="ExternalInput") if has_b1 else None

    lg_d = nc.dram_tensor("o_logits", [BL, OUT], FP32, kind="ExternalOutput")
    # folded state outputs [128, 128]: p=32j+b, col=h'; host unfolds.
    oh_d = nc.dram_tensor("o_h", [2, 128, 128], FP32, kind="ExternalOutput")
    oc_d = nc.dram_tensor("o_c", [2, 128, 128], FP32, kind="ExternalOutput")

    with tile.TileContext(nc) as tc:
        import contextlib
        with contextlib.ExitStack() as ctx:
            wpool = ctx.enter_context(tc.tile_pool(name="weights", bufs=1))
            spool = ctx.enter_context(tc.tile_pool(name="state", bufs=2))
            gpool = ctx.enter_context(tc.tile_pool(name="gates", bufs=2))
            zpool = ctx.enter_context(
                tc.tile_pool(name="zpsum", bufs=3, space="PSUM"))
            tpool = ctx.enter_context(
                tc.tile_pool(name="tpsum", bufs=4, space="PSUM"))

            # ---- load weights/inputs to SBUF (resident) ----
            xT = wpool.tile([F, Tsteps * BL], BF16)
            nc.sync.dma_start(xT[:], xT_d[:])
            U0 = wpool.tile([128, KC * 2048], BF16)
            nc.sync.dma_start(U0[:], U0_d[:])
            W1 = wpool.tile([128, KC * 2048], BF16)
            nc.sync.dma_start(W1[:], W1_d[:])
            U1 = wpool.tile([128, KC * 2048], BF16)
            nc.sync.dma_start(U1[:], U1_d[:])
            W0 = wpool.tile([F, 2048], BF16)
            nc.sync.dma_start(W0[:], W0_d[:])
            Wd = wpool.tile([128, KC], BF16)
            nc.sync.dma_start(Wd[:], Wd_d[:])
            ident = wpool.tile([128, 128], FP32)
            nc.sync.dma_start(ident[:], id_d[:])
            b0 = b1 = ones1 = None
            if has_b0 or has_b1:
                ones1 = wpool.tile([1, BL], BF16)
                nc.vector.memset(ones1[:], 1.0)
            if has_b0:
                b0 = wpool.tile([1, 2048], BF16)
                nc.sync.dma_start(b0[:], b0_d[:])
            if has_b1:
                b1 = wpool.tile([1, 2048], BF16)
                nc.sync.dma_start(b1[:], b1_d[:])

            # ---- initial state: zeros ----
            h0T = spool.tile([128, 128], BF16, tag="h0T")
            h1T = spool.tile([128, 128], BF16, tag="h1T")
            c0 = spool.tile([128, 128], FP32, tag="c0f")
            c1 = spool.tile([128, 128], FP32, tag="c1f")
            for st in (h0T, h1T, c0, c1):
                nc.vector.memset(st[:], 0.0)

            def gemm_chunks(z, stT, Wp, first, last, bias=None):
                """Accumulate st @ W into z for all 4 col-tiles.

                stT: bf16 [128, 128] transposed state (col 32k+b), or
                     (xT, t) tuple for the K=32 input chunk.
                Wp:  bf16 [128, KC*2048] permuted weights; rhs for (k, j) is
                     Wp[:, k*2048 + j*512 : k*2048 + (j+1)*512].
                """
                for j in range(NJ):
                    zj = z[32 * j:32 * (j + 1), :]
                    if bias is not None:
                        nc.tensor.matmul(
                            zj, ones1[:], bias[:, 512 * j:512 * (j + 1)],
                            start=first, stop=False, tile_position=(0, 32 * j))
                    if isinstance(stT, tuple):
                        xt, t = stT
                        nc.tensor.matmul(
                            zj, xt[:, t * BL:(t + 1) * BL],
                            Wp[:, 512 * j:512 * (j + 1)],
                            start=(first and bias is None), stop=False,
                            tile_position=(0, 32 * j))
                    else:
                        for k in range(KC):
                            nc.tensor.matmul(
                                zj, stT[:, 32 * k:32 * (k + 1)],
                                Wp[:, 2048 * k + 512 * j: 2048 * k + 512 * (j + 1)],
                                start=(first and bias is None and k == 0),
                                stop=(last and k == KC - 1),
                                tile_position=(0, 32 * j))

            def lstm_gates(z, cprev, ctag):
                """PSUM z [128, 512] (cols: i,f,o,g) -> (c_new, h_new) fp32."""
                ifo = gpool.tile([128, 384], FP32, tag="gifo")
                nc.scalar.activation(ifo[:], z[:, 0:384], AFT.Sigmoid)
                gg = gpool.tile([128, 128], FP32, tag="gg")
                nc.scalar.activation(gg[:], z[:, 384:512], AFT.Tanh)
                cn = spool.tile([128, 128], FP32, tag=ctag)
                tmp = gpool.tile([128, 128], FP32, tag="gt")
                nc.vector.tensor_mul(tmp[:], ifo[:, 0:128], gg[:])
                nc.vector.tensor_mul(cn[:], ifo[:, 128:256], cprev[:])
                nc.vector.tensor_add(cn[:], cn[:], tmp[:])
                th = gpool.tile([128, 128], FP32, tag="gth")
                nc.scalar.activation(th[:], cn[:], AFT.Tanh)
                hn = spool.tile([128, 128], FP32, tag="hf")
                nc.vector.tensor_mul(hn[:], ifo[:, 256:384], th[:])
                return cn, hn

            def transpose_h(hn, tag, eng):
                """folded h fp32 [128,128] -> hT bf16 [128,128] (col 32k+b).

                One full 128x128 PE transpose: out[h', 32j+b] = hn[32j+b, h'],
                whose [:, 32k:32k+32] slices are exactly the K-chunk
                stationaries for the next step's GEMM.
                """
                hT = spool.tile([128, 128], BF16, tag=tag)
                if tmode == "t1":
                    tp = tpool.tile([128, 128], FP32, tag="tp")
                    nc.tensor.transpose(tp[:], hn[:], ident[:])
                    if eng is nc.vector:
                        nc.vector.tensor_copy(hT[:], tp[:])
                    else:
                        nc.scalar.copy(hT[:], tp[:])
                elif tmode == "t4":  # four row-tiled [32,128] transposes
                    for k in range(KC):
                        tp = tpool.tile([128, 32], FP32, tag="tp")
                        nc.tensor.transpose(
                            tp[:], hn[32 * k:32 * (k + 1), :],
                            ident[32 * k:32 * (k + 1), 32 * k:32 * (k + 1)],
                            tile_position=(32 * k, 0))
                        if eng is nc.vector:
                            nc.vector.tensor_copy(
                                hT[:, 32 * k:32 * (k + 1)], tp[:])
                        else:
                            nc.scalar.copy(hT[:, 32 * k:32 * (k + 1)], tp[:])
                else:  # t4b: 4 transposes into one psum tile, 1 copy
                    tp = tpool.tile([128, 128], FP32, tag="tp")
                    for k in range(KC):
                        nc.tensor.transpose(
                            tp[:, 32 * k:32 * (k + 1)],
                            hn[32 * k:32 * (k + 1), :],
                            ident[32 * k:32 * (k + 1), 32 * k:32 * (k + 1)],
                            tile_position=(32 * k, 0))
                    if eng is nc.vector:
                        nc.vector.tensor_copy(hT[:], tp[:])
                    else:
                        nc.scalar.copy(hT[:], tp[:])
                return hT

            h0n = h1n = None
            sink = None
            if ablate:
                sink = spool.tile([128, 8], FP32, tag="sink")
            for _rep in range(reps):
              for t in range(Tsteps):
                # layer 0: z0 = x_t @ W0 + h0 @ U0 (+ b0)
                z0 = zpool.tile([128, 512], FP32, tag="z")
                gemm_chunks(z0, (xT, t), W0, first=True, last=False, bias=b0)
                gemm_chunks(z0, h0T, U0, first=False, last=True)
                # layer 1 part A: z1 = h1 @ U1 (+ b1)  (independent of h0_new)
                z1 = zpool.tile([128, 512], FP32, tag="z")
                gemm_chunks(z1, h1T, U1, first=True, last=False, bias=b1)
                if ablate == "gemm":
                    nc.scalar.copy(sink[:], z0[:, 0:8])
                    gemm_chunks(z1, h0T, W1, first=False, last=True)
                    nc.scalar.copy(sink[:], z1[:, 0:8])
                    h0n = h1n = c0
                    continue
                # layer 0 gates -> h0_new, then its transpose
                c0, h0n = lstm_gates(z0, c0, "c0f")
                if ablate != "notrans":
                    h0T = transpose_h(h0n, "h0T", nc.scalar)
                else:
                    nc.scalar.copy(sink[:], h0n[:, 0:8])
                # layer 1 part B: z1 += h0_new @ W1
                gemm_chunks(z1, h0T, W1, first=False, last=True)
                c1, h1n = lstm_gates(z1, c1, "c1f")
                if ablate != "notrans":
                    h1T = transpose_h(h1n, "h1T", nc.vector)
                else:
                    nc.scalar.copy(sink[:], h1n[:, 0:8])

            # ---- head: logits = h1 @ Wd (+ bd host-side) ----
            lg_ps = tpool.tile([BL, OUT], FP32, tag="tp")
            for k in range(KC):
                nc.tensor.matmul(lg_ps[:], h1T[:, 32 * k:32 * (k + 1)],
                                 Wd[:, k:k + 1],
                                 start=(k == 0), stop=(k == KC - 1))
            lg = spool.tile([BL, OUT], FP32, tag="lgs")
            nc.scalar.copy(lg[:], lg_ps[:])

            # ---- outputs (folded fp32 states) ----
            nc.sync.dma_start(lg_d[:], lg[:])
            nc.sync.dma_start(oh_d[0], h0n[:])
            nc.sync.dma_start(oh_d[1], h1n[:])
            nc.sync.dma_start(oc_d[0], c0[:])
            nc.sync.dma_start(oc_d[1], c1[:])

    nc.compile()
    return nc


def _permute_w(M):
    """[K, 2048] -> out[:, j*512+t*128+h'] = M[:, g(t)*512+j*128+h'].

    Gate target order t: i,f,o,g (source Keras order i,f,g,o) so one
    sigmoid covers cols 0:384 and tanh covers 384:512.
    """
    Kd = M.shape[0]
    return np.ascontiguousarray(
        M.reshape(Kd, 4, 4, 128)[:, (0, 1, 3, 2)]
        .transpose(0, 2, 1, 3).reshape(Kd, 2048))


def _prep_shared(W0, U0, b0, W1, U1, b1, Wd):
    """Host-side weight rearrangement shared by all cores."""
    bf = ml_dtypes.bfloat16

    def rk(M):  # [H, 2048] -> [128, KC*2048], col = k*2048 + permuted-n
        Mp = _permute_w(M)
        return np.ascontiguousarray(
            Mp.reshape(KC, 128, 2048).transpose(1, 0, 2).reshape(128, KC * 2048)
        ).astype(bf)

    shared = {
        "U0p": rk(U0), "W1p": rk(W1), "U1p": rk(U1),
        "W0p": _permute_w(W0).astype(bf),
        "Wdr": np.ascontiguousarray(Wd.reshape(KC, 128).T).astype(bf),
        "ident128": np.eye(128, dtype=np.float32),
    }
    if np.any(b0):
        shared["b0p"] = _permute_w(b0.reshape(1, 2048)).astype(bf)
    if np.any(b1):
        shared["b1p"] = _permute_w(b1.reshape(1, 2048)).astype(bf)
    return shared


def _unfold_state(a):
    """[128,128] folded (p=32j+b, col h') -> [BL, H] batch-major."""
    return np.ascontiguousarray(
        a.reshape(4, BL, 128).transpose(1, 0, 2).reshape(BL, H))


def kernel(inputs, W0, U0, b0, W1, U1, b1, Wd, bd):
    inputs = np.asarray(inputs, dtype=np.float32)
    W0, U0, b0 = map(np.asarray, (W0, U0, b0))
    W1, U1, b1 = map(np.asarray, (W1, U1, b1))
    Wd, bd = np.asarray(Wd), np.asarray(bd)

    has_b0, has_b1 = bool(np.any(b0)), bool(np.any(b1))
    nc = _build(T, has_b0, has_b1)

    shared = _prep_shared(W0, U0, b0, W1, U1, b1, Wd)
    bf = ml_dtypes.bfloat16
    in_maps = []
    for c in range(NCORES):
        xc = inputs[c * BL:(c + 1) * BL]            # [BL, T, F]
        xT = np.ascontiguousarray(
            xc.transpose(2, 1, 0).reshape(F, T * BL)).astype(bf)
        in_maps.append({"xT": xT, **shared})

    res = run_bass_kernel_spmd(nc, in_maps, list(range(NCORES)))

    logits = np.concatenate([res.results[c]["o_logits"] for c in range(NCORES)],
                            axis=0) + np.float32(bd)[None, :]
    hs = np.stack([
        np.concatenate([_unfold_state(res.results[c]["o_h"][l])
                        for c in range(NCORES)], axis=0) for l in range(2)])
    cs = np.stack([
        np.concatenate([_unfold_state(res.results[c]["o_c"][l])
                        for c in range(NCORES)], axis=0) for l in range(2)])
    return logits.astype(np.float32), hs, cs
